# revision 1
# baseline (speedup 1.0000x reference)
"""Trainium2 Bass kernel for nn_Conv2d_39273180955611.

Conv2d(16->16, 3x3, stride 1, pad 1) applied identically to each of 512
lwe components: x (1,16,64,64,512) -> y (1,16,64,64,512).

Strategy (8 NeuronCores, lwe axis sharded 64 per core; per-core l split
into 8 chunks of 8):
  - Output rows blocked by 6 (11 blocks); each block's 8-row input
    window x (Cin=16) = 128 forms the PE contraction dim. lhsT[dw] is a
    [128, 128] block-banded matrix (cols 96:128 zero-padded so Fast
    Weight Load engages): row (hj,ci), col (ho,co) nonzero iff
    kh = hj-ho in {0,1,2}.
  - rhs is a [128, 64w x 8l = 512] shifted slice of the im2row-prepped
    fp16 input; 3 matmuls (dw=0,1,2) accumulate one PSUM bank [128,512].
  - dw-OUTER sweeps over groups of 8 PSUM banks: the stationary lhsT is
    identical across each 8-matmul sweep, so the per-matmul LDWEIGHTS is
    elided/hidden and the PE streams at ~101ns per 512-col fp16 matmul
    (vs ~284ns when the stationary rotates every matmul).
  - ACT/DVE alternate evicting PSUM +bias into fp16 staging tiles.
  - Input DMA on the SP HWDGE ring (nc.sync), output DMA on the ACT ring
    (nc.scalar) so loads prefetch independently of store readiness; 8
    input buffers so every load's buffer-reuse dependency is many chunks
    stale and the prefetch stream never stalls (keeps the PE fed
    back-to-back, avoiding HAM clock-gate re-throttling).
  - Host pre/post: shard + im2row layout + reassembly (numpy).
"""

import numpy as np

import concourse.bass as bass
import concourse.mybir as mybir
import concourse.tile as tile
from concourse.bass_utils import run_bass_kernel_spmd

NCORES = 8
NCHUNK = 8       # l-chunks per core
LC = 8           # l per chunk -> N = 64w*8l = 512
NB = 11          # h blocks
BH = 6           # output rows per block
WIN = 8          # input rows per window
WP = 66          # padded width
CIN = 16
COUT = 16
LSH = 64         # l per core

MM_DT = mybir.dt.float16
OUT_DT = mybir.dt.float16


def _legalize_waits(nc, max_waits=1):
    """This walrus snapshot rejects >1 sync-wait per instruction; split
    extras onto same-engine NoOps inserted just before."""
    ctr = 0
    for f in nc.m.functions:
        for blk in f.blocks:
            insts = blk.instructions
            i = 0
            while i < len(insts):
                inst = insts[i]
                si = inst.sync_info
                nw = len(si.on_wait) if si is not None else 0
                if nw > max_waits:
                    waits = list(si.on_wait)
                    keep, spill = waits[-max_waits:], waits[:-max_waits]
                    nops = []
                    for w in spill:
                        nop = mybir.InstNoOp(name=f"waitsplit_{ctr}",
                                             engine=inst.engine)
                        ctr += 1
                        nop.sync_info = mybir.SyncInfo(on_wait=[w], on_update=[])
                        nops.append(nop)
                    inst.sync_info = mybir.SyncInfo(on_wait=keep,
                                                    on_update=list(si.on_update))
                    insts[i:i] = nops
                    i += len(nops)
                i += 1
    return ctr


def build_nc(repeat=0, internal_io=False):
    """Build the per-core Bass program (same program on all 8 cores)."""
    in_dt = MM_DT
    nc = bass.Bass("TRN2", target_bir_lowering=False, debug=False,
                   num_devices=1)
    io_kind = "Internal" if internal_io else None
    xin_d = nc.dram_tensor("xprep", [NCHUNK, 128, NB * WP * LC], in_dt,
                           kind=io_kind or "ExternalInput").ap()
    lw_d = nc.dram_tensor("lw", [3, 128, 128], in_dt,
                          kind="ExternalInput").ap()
    bias_d = nc.dram_tensor("biasf", [128, 1], mybir.dt.float32,
                            kind="ExternalInput").ap()
    y_d = nc.dram_tensor("y", [NCHUNK, 96, NB * 512], OUT_DT,
                         kind=io_kind or "ExternalOutput").ap()
    tout_d = None
    if internal_io:
        tout_d = nc.dram_tensor("tout", [128, 1], mybir.dt.float32,
                                kind="ExternalOutput").ap()

    groups = [range(0, 8), range(8, NB)]

    with tile.TileContext(nc) as tc:
        with (
            tc.tile_pool(name="const", bufs=1) as cpool,
            tc.tile_pool(name="xin", bufs=8) as xpool,
            tc.tile_pool(name="yout", bufs=4) as ypool,
            tc.tile_pool(name="ps", bufs=8, space="PSUM") as pspool,
        ):
            lws = []
            for dw in range(3):
                t = cpool.tile([128, 128], in_dt, tag=f"lw{dw}",
                               name=f"lw{dw}")
                nc.sync.dma_start(out=t[:], in_=lw_d[dw])
                lws.append(t)
            bias_t = cpool.tile([128, 1], mybir.dt.float32, tag="bias")
            nc.sync.dma_start(out=bias_t[:], in_=bias_d[:])

            def body():
                for ci in range(NCHUNK):
                    xin = xpool.tile([128, NB * WP * LC], in_dt,
                                     tag="xin")
                    nc.sync.dma_start(out=xin[:], in_=xin_d[ci])
                    xr = xin[:].rearrange("p (b w l) -> p b w l",
                                          b=NB, w=WP, l=LC)
                    ysb = ypool.tile([128, NB * 512], OUT_DT, tag="ysb")

                    for grp in groups:
                        pss = {b: pspool.tile([128, 512],
                                              mybir.dt.float32,
                                              name=f"ps{b}", tag="ps")
                               for b in grp}
                        for dw in range(3):
                            for b in grp:
                                rhs = xr[:, b, dw:dw + 64, :]
                                nc.tensor.matmul(
                                    pss[b][:, :], lws[dw][:], rhs,
                                    start=(dw == 0), stop=(dw == 2),
                                )
                        for i, b in enumerate(grp):
                            yv = ysb[0:96, b * 512:(b + 1) * 512]
                            if i % 2 == 0:
                                nc.scalar.activation(
                                    yv, pss[b][0:96, :],
                                    mybir.ActivationFunctionType.Identity,
                                    bias=bias_t[0:96, :],
                                )
                            else:
                                nc.vector.tensor_scalar_add(
                                    yv, pss[b][0:96, :], bias_t[0:96, :],
                                )
                    nc.scalar.dma_start(out=y_d[ci], in_=ysb[0:96, :])

            if repeat:
                with tc.For_i(0, repeat, 1,
                              hint_engines=(mybir.EngineType.PE,)):
                    body()
            else:
                body()
            if tout_d is not None:
                nc.sync.dma_start(out=tout_d[:], in_=bias_t[:])

    _legalize_waits(nc)
    return nc


def prep_core_inputs(x, weight, bias, core):
    """Host-side shard + im2row prep for one core. x: (1,16,64,64,512)."""
    in_np = np.float16
    xs = x[0, :, :, :, core * LSH:(core + 1) * LSH]          # [ci, h, w, l]
    xpad = np.zeros((CIN, 68, WP, LSH), np.float32)
    xpad[:, 1:65, 1:65, :] = xs
    rows = 6 * np.arange(NB)[None, :] + np.arange(WIN)[:, None]   # [hj, b]
    xp = xpad[:, rows, :, :]                                  # [ci, hj, b, wp, l]
    xp = xp.transpose(1, 0, 2, 3, 4).reshape(128, NB, WP, LSH)
    xp = xp.reshape(128, NB, WP, NCHUNK, LC).transpose(3, 0, 1, 2, 4)
    xprep = np.ascontiguousarray(
        xp.reshape(NCHUNK, 128, NB * WP * LC)).astype(in_np)

    # lhsT[dw, hj*16+ci, ho*16+co] = weight[co, ci, hj-ho, dw]; cols
    # 96:128 stay zero (FWL padding)
    lw = np.zeros((3, 128, 128), np.float32)
    for hj in range(WIN):
        for ho in range(BH):
            kh = hj - ho
            if 0 <= kh <= 2:
                lw[:, hj * 16:(hj + 1) * 16, ho * 16:(ho + 1) * 16] = \
                    weight[:, :, kh, :].transpose(2, 1, 0)
    lw = lw.astype(in_np)

    biasf = np.zeros((128, 1), np.float32)
    biasf[:96, 0] = np.tile(bias, BH)
    return {"xprep": xprep, "lw": lw, "biasf": biasf}


def assemble_core_output(y_core):
    """y_core: [NCHUNK, 96, NB*512] -> [co, h, w, l] (64 rows)."""
    y_core = np.asarray(y_core, dtype=np.float32)
    yc = y_core.reshape(NCHUNK, BH, COUT, NB, 64, LC)
    yc = yc.transpose(2, 3, 1, 4, 0, 5)          # [co, b, ho, w, lc, l8]
    yc = yc.reshape(COUT, NB * BH, 64, LSH)[:, :64]
    return yc


_NC_CACHE = {}


def kernel(x, weight, bias):
    x = np.asarray(x, dtype=np.float32)
    weight = np.asarray(weight, dtype=np.float32)
    bias = np.asarray(bias, dtype=np.float32)

    if "nc" not in _NC_CACHE:
        _NC_CACHE["nc"] = build_nc()
    nc = _NC_CACHE["nc"]

    in_maps = [prep_core_inputs(x, weight, bias, c) for c in range(NCORES)]
    res = run_bass_kernel_spmd(nc, in_maps, core_ids=list(range(NCORES)))

    y = np.empty((1, 16, 64, 64, 512), np.float32)
    for c in range(NCORES):
        y[0, :, :, :, c * LSH:(c + 1) * LSH] = \
            assemble_core_output(res.results[c]["y"])
    return y



# revision 3
# speedup vs baseline: 1.1121x; 1.1121x over previous
"""Trainium2 Bass kernel for nn_Conv2d_39273180955611.

Conv2d(16->16, 3x3, stride 1, pad 1) applied identically to each of 512
lwe components: x (1,16,64,64,512) -> y (1,16,64,64,512).

Strategy (8 NeuronCores, lwe axis sharded 64 per core; per-core l split
into 8 chunks of 8):
  - Output rows blocked by 6 (11 blocks); each block's 8-row input
    window x (Cin=16) = 128 forms the PE contraction dim. lhsT[dw] is a
    [128, 128] block-banded matrix (cols 96:128 zero-padded so Fast
    Weight Load engages): row (hj,ci), col (ho,co) nonzero iff
    kh = hj-ho in {0,1,2}.
  - rhs is a [128, 64w x 8l = 512] shifted slice of the im2row-prepped
    fp16 input; 3 matmuls (dw=0,1,2) accumulate one PSUM bank [128,512].
  - dw-OUTER sweeps over groups of 8 PSUM banks: the stationary lhsT is
    identical across each 8-matmul sweep, so the per-matmul LDWEIGHTS is
    elided/hidden and the PE streams at ~101ns per 512-col fp16 matmul
    (vs ~284ns when the stationary rotates every matmul).
  - ACT/DVE alternate evicting PSUM +bias into fp16 staging tiles.
  - ALL HBM DMA (loads AND stores) on the single SP HWDGE ring
    (nc.sync): one ring alternates ~MB-scale read bursts and write
    bursts, which measures ~8% faster than loads-on-SP/stores-on-ACT
    (63.1 vs 68.9 us for the I/O stream alone) -- two rings interleave
    reads/writes finely at the HBM controller. To avoid head-of-line
    blocking (a store's eviction-wait would stall later loads in the
    FIFO ring), stores are software-pipelined: chunk c's store is
    issued Delta=3 chunks later, when its evictions are long done.
    Delta=2 stalls (76us), extra staging bufs hurt (74us); Delta=3 with
    4 staging bufs hits the DMA floor (63.8us vs 73.3 two-ring, same
    process).
  - Measured floors (same method): DMA-only single-ring 63.1us, PE-only
    59.2us (264 matmuls -> 224ns per 512-col fp16 matmul; the hw
    streams 1 col/cycle at 2.4GHz, so dw-outer sweeps mostly hide
    LDWEIGHTS), full kernel 63.8us = DMA floor + 0.7us. fp8/e4m3 paths
    are dead: rel err 3.9e-2 > 2e-2 tolerance. Input im2row row
    duplication (88/64 slots) is structurally unavoidable: SBUF->SBUF
    dedup copies cost 473us (tiny-DMA serialization), partial-matmul
    dedup costs 2x PE passes, and trimming pad rows/cols via split DMAs
    loses more to per-transfer overhead than the 7.6% byte savings.
  - Host pre/post: shard + im2row layout + reassembly (numpy).
"""

import numpy as np

import concourse.bass as bass
import concourse.mybir as mybir
import concourse.tile as tile
from concourse.bass_utils import run_bass_kernel_spmd

NCORES = 8
NCHUNK = 8       # l-chunks per core
LC = 8           # l per chunk -> N = 64w*8l = 512
NB = 11          # h blocks
BH = 6           # output rows per block
WIN = 8          # input rows per window
WP = 66          # padded width
CIN = 16
COUT = 16
LSH = 64         # l per core

MM_DT = mybir.dt.float16
OUT_DT = mybir.dt.float16


def _legalize_waits(nc, max_waits=1):
    """This walrus snapshot rejects >1 sync-wait per instruction; split
    extras onto same-engine NoOps inserted just before."""
    ctr = 0
    for f in nc.m.functions:
        for blk in f.blocks:
            insts = blk.instructions
            i = 0
            while i < len(insts):
                inst = insts[i]
                si = inst.sync_info
                nw = len(si.on_wait) if si is not None else 0
                if nw > max_waits:
                    waits = list(si.on_wait)
                    keep, spill = waits[-max_waits:], waits[:-max_waits]
                    nops = []
                    for w in spill:
                        nop = mybir.InstNoOp(name=f"waitsplit_{ctr}",
                                             engine=inst.engine)
                        ctr += 1
                        nop.sync_info = mybir.SyncInfo(on_wait=[w], on_update=[])
                        nops.append(nop)
                    inst.sync_info = mybir.SyncInfo(on_wait=keep,
                                                    on_update=list(si.on_update))
                    insts[i:i] = nops
                    i += len(nops)
                i += 1
    return ctr


def build_nc(repeat=0, internal_io=False):
    """Build the per-core Bass program (same program on all 8 cores)."""
    in_dt = MM_DT
    nc = bass.Bass("TRN2", target_bir_lowering=False, debug=False,
                   num_devices=1)
    io_kind = "Internal" if internal_io else None
    xin_d = nc.dram_tensor("xprep", [NCHUNK, 128, NB * WP * LC], in_dt,
                           kind=io_kind or "ExternalInput").ap()
    lw_d = nc.dram_tensor("lw", [3, 128, 128], in_dt,
                          kind="ExternalInput").ap()
    bias_d = nc.dram_tensor("biasf", [128, 1], mybir.dt.float32,
                            kind="ExternalInput").ap()
    y_d = nc.dram_tensor("y", [NCHUNK, 96, NB * 512], OUT_DT,
                         kind=io_kind or "ExternalOutput").ap()
    tout_d = None
    if internal_io:
        tout_d = nc.dram_tensor("tout", [128, 1], mybir.dt.float32,
                                kind="ExternalOutput").ap()

    groups = [range(0, 8), range(8, NB)]

    with tile.TileContext(nc) as tc:
        with (
            tc.tile_pool(name="const", bufs=1) as cpool,
            tc.tile_pool(name="xin", bufs=8) as xpool,
            tc.tile_pool(name="yout", bufs=4) as ypool,
            tc.tile_pool(name="ps", bufs=8, space="PSUM") as pspool,
        ):
            lws = []
            for dw in range(3):
                t = cpool.tile([128, 128], in_dt, tag=f"lw{dw}",
                               name=f"lw{dw}")
                nc.sync.dma_start(out=t[:], in_=lw_d[dw])
                lws.append(t)
            bias_t = cpool.tile([128, 1], mybir.dt.float32, tag="bias")
            nc.sync.dma_start(out=bias_t[:], in_=bias_d[:])

            DELTA = 3   # store-issue lag (chunks) on the shared ring

            def body():
                pend = []
                for ci in range(NCHUNK):
                    if pend and pend[0][0] <= ci - DELTA:
                        ci_s, ysb_s = pend.pop(0)
                        nc.sync.dma_start(out=y_d[ci_s],
                                          in_=ysb_s[0:96, :])
                    xin = xpool.tile([128, NB * WP * LC], in_dt,
                                     tag="xin")
                    nc.sync.dma_start(out=xin[:], in_=xin_d[ci])
                    xr = xin[:].rearrange("p (b w l) -> p b w l",
                                          b=NB, w=WP, l=LC)
                    ysb = ypool.tile([128, NB * 512], OUT_DT, tag="ysb")

                    for grp in groups:
                        pss = {b: pspool.tile([128, 512],
                                              mybir.dt.float32,
                                              name=f"ps{b}", tag="ps")
                               for b in grp}
                        for dw in range(3):
                            for b in grp:
                                rhs = xr[:, b, dw:dw + 64, :]
                                nc.tensor.matmul(
                                    pss[b][:, :], lws[dw][:], rhs,
                                    start=(dw == 0), stop=(dw == 2),
                                )
                        for i, b in enumerate(grp):
                            yv = ysb[0:96, b * 512:(b + 1) * 512]
                            if i % 2 == 0:
                                nc.scalar.activation(
                                    yv, pss[b][0:96, :],
                                    mybir.ActivationFunctionType.Identity,
                                    bias=bias_t[0:96, :],
                                )
                            else:
                                nc.vector.tensor_scalar_add(
                                    yv, pss[b][0:96, :], bias_t[0:96, :],
                                )
                    pend.append((ci, ysb))
                for ci_s, ysb_s in pend:
                    nc.sync.dma_start(out=y_d[ci_s], in_=ysb_s[0:96, :])

            if repeat:
                with tc.For_i(0, repeat, 1,
                              hint_engines=(mybir.EngineType.PE,)):
                    body()
            else:
                body()
            if tout_d is not None:
                nc.sync.dma_start(out=tout_d[:], in_=bias_t[:])

    _legalize_waits(nc)
    return nc


def prep_core_inputs(x, weight, bias, core):
    """Host-side shard + im2row prep for one core. x: (1,16,64,64,512)."""
    in_np = np.float16
    xs = x[0, :, :, :, core * LSH:(core + 1) * LSH]          # [ci, h, w, l]
    xpad = np.zeros((CIN, 68, WP, LSH), np.float32)
    xpad[:, 1:65, 1:65, :] = xs
    rows = 6 * np.arange(NB)[None, :] + np.arange(WIN)[:, None]   # [hj, b]
    xp = xpad[:, rows, :, :]                                  # [ci, hj, b, wp, l]
    xp = xp.transpose(1, 0, 2, 3, 4).reshape(128, NB, WP, LSH)
    xp = xp.reshape(128, NB, WP, NCHUNK, LC).transpose(3, 0, 1, 2, 4)
    xprep = np.ascontiguousarray(
        xp.reshape(NCHUNK, 128, NB * WP * LC)).astype(in_np)

    # lhsT[dw, hj*16+ci, ho*16+co] = weight[co, ci, hj-ho, dw]; cols
    # 96:128 stay zero (FWL padding)
    lw = np.zeros((3, 128, 128), np.float32)
    for hj in range(WIN):
        for ho in range(BH):
            kh = hj - ho
            if 0 <= kh <= 2:
                lw[:, hj * 16:(hj + 1) * 16, ho * 16:(ho + 1) * 16] = \
                    weight[:, :, kh, :].transpose(2, 1, 0)
    lw = lw.astype(in_np)

    biasf = np.zeros((128, 1), np.float32)
    biasf[:96, 0] = np.tile(bias, BH)
    return {"xprep": xprep, "lw": lw, "biasf": biasf}


def assemble_core_output(y_core):
    """y_core: [NCHUNK, 96, NB*512] -> [co, h, w, l] (64 rows)."""
    y_core = np.asarray(y_core, dtype=np.float32)
    yc = y_core.reshape(NCHUNK, BH, COUT, NB, 64, LC)
    yc = yc.transpose(2, 3, 1, 4, 0, 5)          # [co, b, ho, w, lc, l8]
    yc = yc.reshape(COUT, NB * BH, 64, LSH)[:, :64]
    return yc


_NC_CACHE = {}


def kernel(x, weight, bias):
    x = np.asarray(x, dtype=np.float32)
    weight = np.asarray(weight, dtype=np.float32)
    bias = np.asarray(bias, dtype=np.float32)

    if "nc" not in _NC_CACHE:
        _NC_CACHE["nc"] = build_nc()
    nc = _NC_CACHE["nc"]

    in_maps = [prep_core_inputs(x, weight, bias, c) for c in range(NCORES)]
    res = run_bass_kernel_spmd(nc, in_maps, core_ids=list(range(NCORES)))

    y = np.empty((1, 16, 64, 64, 512), np.float32)
    for c in range(NCORES):
        y[0, :, :, :, c * LSH:(c + 1) * LSH] = \
            assemble_core_output(res.results[c]["y"])
    return y

